# revision 80
# baseline (speedup 1.0000x reference)
"""Trainium2 Bass kernel for nn_Attention_sep (separate patch/det QKV attention).

Sharding: query rows split across 8 cores (528 patch + 16 det queries per
core, zero-padded); K/V projections replicated per core. All SBUF tensors are
bf16 (PSUM accumulation fp32), which fits K^T and V fully in SBUF (no DRAM
round-trip), runs every matmul at 1 cycle/row regardless of moving size, and
halves DMA traffic. Per core, per x-chunk (8x512 + 105 + 100 tokens): K^T and
token-major V (+ ones column for sumexp) are projected into per-chunk SBUF
tiles; attention streams right behind: per (head, 128-key chunk) S^T = K_h^T'
Q_h^T into one PSUM bank (512 main queries), exp on ScalarE straight from
PSUM into bf16 pt, then token-major attn@V (stationary pt 128-query blocks,
moving V[kc,66]) accumulates o[q, 64hd+sumexp] in four PSUM banks across the
x-chunk, flushed-added to an SBUF fp32 accumulator. Heads run in two sextets
so PSUM fits (2 proj + 2 ps + 4 po banks = 8). The last 32 queries run as a
separate o^T-layout mini-attention afterwards. Tail: divide by sumexp,
LayerNorm (bn_stats/bn_aggr, exact eps), PE transpose to feature-major, and
the patch/det output projections (fp32 output).

Host only slices/transposes/casts inputs and gathers per-core outputs.
Dispatch uploads shared inputs sharded (1x wire) and replicates them
on-device; replicated weights are cached across calls.
"""
import sys
sys.path.insert(0, "/opt/trn_rl_repo")
import numpy as np

N_TOK = 4301
NPAD = 4320
D = 768
H = 12
HD = 64
NDET = 100
NPATCH = N_TOK - NDET          # 4201
SCALE = HD ** -0.5
EPS = 1e-5
NCORES = 8
PQ = 528                        # per-core patch queries (528*8 = 4224 >= 4201)
DQ = 16                         # per-core det queries (16*8 = 128 >= 100)
TQ = PQ + DQ                    # 544
MQ = 512                        # main query block (4 x 128)
TLQ = TQ - MQ                   # 32 tail queries
DC = D // 128                   # 6 feature/contraction chunks

# x / key chunking: 8 x 512-token x-chunks (4 key chunks each) + 105 + 100
XN_STARTS = [512 * i for i in range(8)] + [4096, 4201]
XN_SIZES = [512] * 8 + [105, 100]
XN_PAD = [512] * 8 + [112, 112]        # 16B-aligned strides (dual-fp8 lw)
KC_STARTS = [128 * i for i in range(32)] + [4096, 4201]
KC_SIZES = [128] * 32 + [105, 100]
NKC = len(KC_SIZES)             # 34
XC_CHUNKS = [list(range(4 * i, 4 * i + 4)) for i in range(8)] + [[32], [33]]

_CACHE = {}


def _build(phases=5):
    import concourse.bass as bass
    import concourse.tile as tile
    from concourse import bacc, mybir
    from concourse.masks import make_identity

    FP32 = mybir.dt.float32
    BF16 = mybir.dt.bfloat16
    FP8 = mybir.dt.float8e4
    AF = mybir.ActivationFunctionType
    ALU = mybir.AluOpType
    DR = mybir.MatmulPerfMode.DoubleRow

    nc = bacc.Bacc(name="attn_sep")

    def din(name, shape, dt=BF16):
        return nc.dram_tensor(name, shape, dt, kind="ExternalInput")

    xT = din("xT", [D, NPAD])
    xqT = din("xqT", [D, TQ])
    w_in = {k: din(k, [D, D]) for k in
            ["wqT_p", "wqT_d", "wkT_p", "wkT_d", "wvT_p", "wvT_d",
             "woT_p", "woT_d"]}
    b_in = {k: din(k, [D], FP32) for k in
            ["bq_p", "bq_d", "bv_p", "bv_d", "bo_p", "bo_d"]}
    lng = din("lng", [D])
    lnb = din("lnb", [D])
    outT = nc.dram_tensor("outT", [D, TQ], FP32, kind="ExternalOutput")
    outT_v = outT.rearrange("(c p) q -> p c q", p=128)
    xT_v = xT.rearrange("(c p) n -> p c n", p=128)
    xqT_v = xqT.rearrange("(c p) n -> p c n", p=128)

    from contextlib import ExitStack
    with tile.TileContext(nc) as tc:
        with ExitStack() as ctx:
            ep = ctx.enter_context
            sgl = ep(tc.tile_pool(name="sgl", bufs=1))
            wp = ep(tc.tile_pool(name="wp", bufs=4))
            xp = ep(tc.tile_pool(name="xp", bufs=2))
            ktp = ep(tc.tile_pool(name="ktp", bufs=1))
            vtp = ep(tc.tile_pool(name="vtp", bufs=1))
            qtp = ep(tc.tile_pool(name="qtp", bufs=1))
            ptp = ep(tc.tile_pool(name="ptp", bufs=2))
            oap = ep(tc.tile_pool(name="oap", bufs=1))
            onp = ep(tc.tile_pool(name="onp", bufs=1))
            olp = ep(tc.tile_pool(name="olp", bufs=1))
            oup = ep(tc.tile_pool(name="oup", bufs=2))
            sml = ep(tc.tile_pool(name="sml", bufs=4))
            projp = ep(tc.tile_pool(name="projp", bufs=2, space="PSUM"))
            psp = ep(tc.tile_pool(name="psp", bufs=2, space="PSUM"))
            pop = ep(tc.tile_pool(name="pop", bufs=1, space="PSUM"))

            _prn = [0]

            def prj():
                _prn[0] += 1
                return projp.tile([128, 512], FP32, tag="proj",
                                  name=f"prj_{_prn[0]}")

            # ---- constants / broadcast tiles ----
            ident = sgl.tile([128, 128], BF16, tag="ident")
            make_identity(nc, ident)

            def bcast(src, dt, tag):
                t = sgl.tile([128, D], dt, tag=tag)
                s = src[:]
                nc.gpsimd.dma_start(
                    out=t,
                    in_=bass.AP(tensor=s.tensor, offset=s.offset,
                                ap=[[0, 128]] + [list(a) for a in s.ap]))
                return t

            def perpart(name):
                t = sgl.tile([128, DC], FP32, tag=f"pp_{name}")
                nc.gpsimd.dma_start(t, b_in[name].rearrange("(c p) -> p c", p=128))
                return t

            eps_t = sgl.tile([128, 1], FP32, tag="eps")
            nc.vector.memset(eps_t, EPS)

            def load_w(name, eng):
                t = wp.tile([128, DC, D], BF16, tag="w")
                eng.dma_start(t, w_in[name].rearrange("(c p) f -> p c f", p=128))
                return t

            # ---- resident tensors ----
            # Q/K live only in the S matmuls, in fp8e4 (baseline feature
            # layout: head h at partitions 64*(h%2) of chunk fc=h//2). The S
            # matmul runs DoubleRow with the two k-tiles used as PRECISION
            # LEVELS: moving = (q8, q_residual8), stationary = k8 broadcast
            # (stride-0), so S = (q8+qr)·k8 — full-precision Q against fp8 K
            # at 2x matmul rate (one-sided 2.4% error instead of two-sided).
            QTP8 = qtp.tile([128, DC, 2, TQ], FP8, tag="qtp8", name="qtp8")
            KT8 = [ktp.tile([128, DC, XN_PAD[xc]], FP8, tag=f"kt{xc}",
                            name=f"kt{xc}")
                   for xc in range(10)]
            VT = [vtp.tile([128, H, 66], BF16, tag=f"vt{c}", name=f"vt{c}")
                  for c in range(NKC)]
            for c in range(NKC):
                nc.vector.memset(VT[c][:, :, 64:65], 1.0)
                nc.vector.memset(VT[c][:, :, 65:66], 0.0)
            o_acc = oap.tile([128, 4, H, 65], FP32, tag="oacc")
            o_n = onp.tile([128, 4, D], BF16, tag="on")
            o_nt = onp.tile([128, D], BF16, tag="ont")

            # ====== weight/x DMAs ordered for earliest first-S ======
            # xq rides the scalar queue (with the xt chunks) so the sync
            # queue reaches wq_p sooner; wk/wq_p land their fc0/fc1 slices
            # first so the hp0 S matmuls only wait ~2 small transfers.
            def load_w_split(name, eng):
                t = wp.tile([128, DC, D], BF16, tag="w")
                v = w_in[name].rearrange("(c p) f -> p c f", p=128)
                eng.dma_start(t[:, :, :256], v[:, :, :256])
                eng.dma_start(t[:, :, 256:], v[:, :, 256:])
                return t

            wk = load_w_split("wkT_p", nc.sync)
            xq = olp.tile([128, DC, TQ], BF16, tag="olnT", name="xq")
            nc.scalar.dma_start(xq, xqT_v)
            wq_p = load_w_split("wqT_p", nc.sync)
            bq_p_s = perpart("bq_p")
            bq_d_s = perpart("bq_d")
            wv = load_w("wvT_p", nc.sync)
            wq_d = load_w("wqT_d", nc.sync)
            bv_p_b = bcast(b_in["bv_p"], FP32, "bc_bvp")
            q_segs = [(0, MQ, wq_p, bq_p_s), (MQ, PQ - MQ, wq_p, bq_p_s),
                      (PQ, DQ, wq_d, bq_d_s)]

            def emit_q_pass(fc, si):
                c0, n, wq, bq = q_segs[si]
                pq = prj()
                for dc in range(DC):
                    nc.tensor.matmul(
                        pq[:, :n],
                        wq[:, dc, 128 * fc:128 * (fc + 1)],
                        xq[:, dc, c0:c0 + n],
                        start=(dc == 0), stop=(dc == DC - 1))
                qtmp = sml.tile([128, 512], BF16, tag="qtmp")
                nc.vector.tensor_scalar_add(
                    qtmp[:, :n], pq[:, :n], bq[:, fc:fc + 1])
                nc.vector.tensor_copy(
                    QTP8[:, fc, 0, c0:c0 + n], qtmp[:, :n])
                nc.vector.tensor_tensor(
                    QTP8[:, fc, 1, c0:c0 + n], qtmp[:, :n],
                    QTP8[:, fc, 0, c0:c0 + n], ALU.subtract)

            # remaining consts + deferred weight loads; FIFO slot order makes
            # wk_d/wv_d land right when wk/wv retire (x-chunk 8) and wo_p/wo_d
            # prefetch into the slots wq_p/wq_d free after the Q projection.
            g_b = bcast(lng, BF16, "bc_g")
            b_b = bcast(lnb, BF16, "bc_b")
            bo_p_s = perpart("bo_p")
            bo_d_s = perpart("bo_d")
            bv_d_b = bcast(b_in["bv_d"], FP32, "bc_bvd")
            wk_d = load_w("wkT_d", nc.scalar)
            wv_d = load_w("wvT_d", nc.scalar)
            wo_p = load_w("woT_p", nc.sync)
            wo_d = load_w("woT_d", nc.sync)

            # =========== streamed K/V projection + main attention ===========
            # Software-pipelined: the K/V projection of x-chunk xc+1 is
            # emitted in slices BETWEEN the head-pair passes of x-chunk xc's
            # attention, so the PE fills its exp-wait bubbles with proj work
            # instead of serializing a 15us ACT-idle proj phase per x-chunk.
            def emit_k_pass(xc, fc, xt):
                szp = XN_PAD[xc]
                wk_x = wk_d if xc == 9 else wk
                pk = prj()
                for dc in range(DC):
                    nc.tensor.matmul(
                        pk[:, :szp],
                        wk_x[:, dc, 128 * fc:128 * (fc + 1)],
                        xt[:, dc, :szp],
                        start=(dc == 0), stop=(dc == DC - 1))
                nc.vector.tensor_copy(
                    KT8[xc][:, fc, :szp], pk[:, :szp])

            def emit_v_half(xc, ci, half, xt):
                sz = XN_SIZES[xc]
                wv_x = wv_d if xc == 9 else wv
                bvb = bv_d_b if xc == 9 else bv_p_b
                c = XC_CHUNKS[xc][ci]
                s0 = 128 * ci
                m = min(128, sz - s0)
                f0 = 384 * half
                pv = prj()
                for dc in range(DC):
                    nc.tensor.matmul(
                        pv[:m, :384],
                        xt[:, dc, s0:s0 + m],
                        wv_x[:, dc, f0:f0 + 384],
                        start=(dc == 0), stop=(dc == DC - 1))
                nc.vector.tensor_tensor(
                    VT[c][:m, 6 * half:6 * (half + 1), :HD],
                    pv[:m, :384].rearrange("p (h d) -> p h d", d=HD),
                    bvb[:m, f0:f0 + 384].rearrange("p (h d) -> p h d", d=HD),
                    ALU.add)

            def load_xt(xc):
                n0, szp = XN_STARTS[xc], XN_PAD[xc]
                xt = xp.tile([128, DC, 512], BF16, tag="x")
                nc.scalar.dma_start(xt[:, :, :szp], xT_v[:, :, n0:n0 + szp])
                return xt

            def proj_units(xc, xt):
                return ([lambda fc=fc: emit_k_pass(xc, fc, xt)
                         for fc in range(DC)] +
                        [lambda ci=ci, hf=hf: emit_v_half(xc, ci, hf, xt)
                         for ci in range(len(XC_CHUNKS[xc]))
                         for hf in range(2)])

            # prologue: xc0 proj + Q passes interleaved for earliest first-S
            # Block-diagonal tail-Q: per (fc, level), col 32b+q holds head
            # (2fc+b)'s tail query q on partitions [64b, 64b+64) and ZERO
            # elsewhere, so the tail S runs as full-128-row matmuls (2 heads
            # per matmul, the zeros killing cross-head terms) — the backend
            # rejects sequences of narrow matmuls whose PE tile config varies.
            QTt8 = sgl.tile([128, DC, 2, 64], FP8, tag="qtt8")
            nc.vector.memset(QTt8, 0.0)

            # minimal prologue: attention(xc0, hp) only needs K/Q pass fc=hp
            # and the V halves, so emit just fc 0-1 + V before the loop and
            # defer the rest (incl. the tail-only Q segs) into the hp slices.
            xt0 = load_xt(0)
            emit_k_pass(0, 0, xt0)
            emit_q_pass(0, 0)
            emit_k_pass(0, 1, xt0)
            emit_q_pass(1, 0)

            def qtt_copies():
                for fc in range(DC):
                    for b in range(2):
                        nc.vector.tensor_copy(
                            QTt8[64 * b:64 * b + 64, fc, :, 32 * b:32 * b + 32],
                            QTP8[64 * b:64 * b + 64, fc, :, MQ:TQ])

            pre_units = []
            for fc in range(2, DC):
                pre_units.append(lambda fc=fc: emit_k_pass(0, fc, xt0))
                pre_units.append(lambda fc=fc: emit_q_pass(fc, 0))
            for fc in range(DC):
                pre_units.append(lambda fc=fc: emit_q_pass(fc, 1))
                pre_units.append(lambda fc=fc: emit_q_pass(fc, 2))
            pre_units.append(qtt_copies)

            for xc in range(10):
                n0 = XN_STARTS[xc]
                pending = proj_units(xc + 1, load_xt(xc + 1)) if xc < 9 else []
                if xc == 0:
                    pending = pre_units + pending
                # attention over this x-chunk's key chunks, head-pair passes
                # (po bank layout: [qb%2 half 256][head-in-pair at 0/85][66])
                chunks = XC_CHUNKS[xc]
                for hp in range(6 if phases >= 2 else 0):
                    take = -(-len(pending) // (6 - hp)) if pending else 0
                    if xc == 0 and hp <= 1:
                        take = 0 if hp == 0 else min(take, 4)
                    for u in pending[:take]:
                        u()
                    pending = pending[take:]
                    po = [pop.tile([128, 512], FP32, tag=f"po{qp}",
                                   name=f"po{qp}_{xc}_{hp}")
                          for qp in range(2)]
                    for cj, c in enumerate(chunks):
                        kc = KC_SIZES[c]
                        kcp = kc + (kc % 2)        # even stationary free dim
                        lk = KC_STARTS[c] - n0
                        ps = psp.tile([128, 2, 512], FP32, tag="ps2")
                        for i in range(2):
                            o64 = 64 * i
                            ks = KT8[xc][o64:o64 + 64, hp, lk:lk + kcp]
                            k2 = bass.AP(tensor=ks.tensor, offset=ks.offset,
                                         ap=[list(ks.ap[0]), [0, 2],
                                             list(ks.ap[1])])
                            nc.tensor.matmul(
                                ps[:kcp, i, :MQ],
                                k2,
                                QTP8[o64:o64 + 64, hp, :, :MQ],
                                start=True, stop=True, perf_mode=DR,
                                tile_position=(o64, 0))
                        pt = ptp.tile([128, 2, 512], BF16, tag="pt")
                        nc.scalar.activation(
                            pt[:kc], ps[:kc], AF.Exp, scale=SCALE)
                        if xc == 0 and hp == 0:
                            # xc0's V projection rides between hp0's chunks so
                            # the first exps aren't stuck behind it in-stream
                            emit_v_half(0, cj, 0, xt0)
                            emit_v_half(0, cj, 1, xt0)
                        # one start/stop per PSUM bank per x-chunk round: the
                        # start lazily zeroes the whole bank, later first
                        # writes to other slots land on pending-zero bytes
                        for i in range(2):
                            h = 2 * hp + i
                            for qb in range(4):
                                nc.tensor.matmul(
                                    po[qb // 2][:, 256 * (qb % 2) + 85 * i:
                                                256 * (qb % 2) + 85 * i + 66],
                                    pt[:kc, i, 128 * qb:128 * (qb + 1)],
                                    VT[c][:kc, h, :],
                                    start=(cj == 0 and i == 0 and qb % 2 == 0),
                                    stop=(cj == len(chunks) - 1 and i == 1
                                          and qb % 2 == 1))
                    for qp in range(2):
                        pv66 = po[qp].rearrange(
                            "p (q s) -> p q s", q=2)[:, :, :170].rearrange(
                            "p q (s r) -> p q s r", r=85)[:, :, :, :65]
                        dst = o_acc[:, 2 * qp:2 * qp + 2, 2 * hp:2 * hp + 2, :]
                        if xc == 0:
                            nc.vector.tensor_copy(dst, pv66)
                        else:
                            nc.vector.tensor_add(dst, dst, pv66)

            # ====== tail 32 queries: token-major, mirrors the main path ======
            # S^T: head h -> ps bank h%2, slot h//2 (6 x 32 cols per bank);
            # attn@V: stationary ptT[kc,32], moving V[kc,66] -> po banks with
            # 6 heads at 85-stride, one start/stop per bank (po0: h<6).
            # ====== LayerNorm helpers (main queries decoupled from tail) =====
            mva = sml.tile([128, 5, 2], FP32, tag="mva")
            rstd = sml.tile([128, 5], FP32, tag="rstd")
            if phases >= 4:
                nc.vector.memset(mva, 1.0)
            if phases >= 5:
                o_lnT = olp.tile([128, DC, TQ], BF16, tag="olnT", name="olnT")
            o_segs = [(0, MQ, wo_p, bo_p_s), (MQ, PQ - MQ, wo_p, bo_p_s),
                      (PQ, DQ, wo_d, bo_d_s)]

            def ln_stats(o_slice, L, col):
                stats = sml.tile([128, 3, 6], FP32, tag="st")
                for gi in range(3):
                    nc.vector.bn_stats(
                        stats[:L, gi], o_slice[:, 256 * gi:256 * (gi + 1)])
                nc.vector.bn_aggr(mva[:L, col], stats[:L])

            def ln_apply(o_slice, L, col):
                nc.vector.tensor_scalar(
                    o_slice, o_slice, mva[:L, col, 0:1], rstd[:L, col:col + 1],
                    ALU.subtract, ALU.mult)
                nc.vector.tensor_tensor(o_slice, o_slice, g_b[:L], ALU.mult)
                nc.vector.tensor_tensor(o_slice, o_slice, b_b[:L], ALU.add)

            def emit_ln_main():
                for qb in range(4):
                    rs = sml.tile([128, H], FP32, tag="rs")
                    nc.vector.reciprocal(rs, o_acc[:, qb, :, 64])
                    rsv = rs[:]
                    rs_b = bass.AP(
                        tensor=rsv.tensor, offset=rsv.offset,
                        ap=[list(rsv.ap[0]), [rsv.ap[1][0], H], [0, HD]])
                    nc.vector.tensor_tensor(
                        o_n[:, qb, :].rearrange("p (h d) -> p h d", d=HD),
                        o_acc[:, qb, :, :HD], rs_b, ALU.mult)
                    ln_stats(o_n[:, qb, :], 128, qb)
                nc.scalar.activation(
                    rstd[:, 0:4], mva[:, 0:4, 1], AF.Sqrt, bias=eps_t)
                nc.vector.reciprocal(rstd[:, 0:4], rstd[:, 0:4])
                for qb in range(4):
                    ln_apply(o_n[:, qb, :], 128, qb)

            def emit_main_T(fc):
                tp = projp.tile([128, 512], BF16, tag="proj",
                                name=f"tpm{fc}")
                for qb in range(4):
                    nc.tensor.transpose(
                        tp[:, 128 * qb:128 * (qb + 1)],
                        o_n[:, qb, 128 * fc:128 * (fc + 1)], ident)
                nc.vector.tensor_copy(o_lnT[:, fc, :MQ], tp)

            def emit_main_O(fc):
                c0, n, wo, bo = o_segs[0]
                pu = prj()
                for dc in range(DC):
                    nc.tensor.matmul(
                        pu[:, :n],
                        wo[:, dc, 128 * fc:128 * (fc + 1)],
                        o_lnT[:, dc, c0:c0 + n],
                        start=(dc == 0), stop=(dc == DC - 1))
                ou = oup.tile([128, MQ], FP32, tag="ou", name=f"oum{fc}")
                nc.vector.tensor_scalar_add(ou, pu[:, :n], bo[:, fc:fc + 1])
                nc.sync.dma_start(outT_v[:, fc, :MQ], ou)

            # main-query LN / transpose / O-proj / out-DMA emitted in slices
            # BETWEEN the tail-attention chunks so they overlap it.
            eu = []
            if phases >= 4:
                eu.append(emit_ln_main)
            if phases >= 5:
                eu += [lambda fc=fc: emit_main_T(fc) for fc in range(DC)]
                eu += [lambda fc=fc: emit_main_O(fc) for fc in range(DC)]

            po_t = [pop.tile([128, 512], FP32, tag=f"po{b}", name=f"pot{b}")
                    for b in range(2)]
            for c in range(NKC if phases >= 3 else 0):
                xc = c // 4 if c < 32 else c - 24
                kc = KC_SIZES[c]
                lk = KC_STARTS[c] - XN_STARTS[xc]
                kcp = kc + (kc % 2)
                psT = psp.tile([128, 2, 512], FP32, tag="ps2")
                for fc in range(DC):
                    for j in range(2):
                        nc.tensor.matmul(
                            psT[:kcp, 0, 64 * fc:64 * fc + 64],
                            KT8[xc][:, fc, lk:lk + kcp],
                            QTt8[:, fc, j, :],
                            start=(fc == 0 and j == 0),
                            stop=(fc == DC - 1 and j == 1))
                ptT = ptp.tile([128, 2, 512], BF16, tag="pt")
                nc.scalar.activation(
                    ptT[:kc, 0, :384], psT[:kc, 0, :384], AF.Exp, scale=SCALE)
                for h in range(H):
                    hs = h % 6
                    col = 64 * (h // 2) + 32 * (h % 2)
                    nc.tensor.matmul(
                        po_t[h // 6][:32, 85 * hs:85 * hs + 66],
                        ptT[:kc, 0, col:col + 32],
                        VT[c][:kc, h, :],
                        start=(c == 0 and hs == 0),
                        stop=(c == NKC - 1 and hs == 5))
                if c % 3 == 2 and eu:
                    eu.pop(0)()
            for u in eu:
                u()
            for h in range(H if phases >= 3 else 0):
                rh = sml.tile([128, 1], FP32, tag="rh")
                src = po_t[h // 6][:32, 85 * (h % 6):85 * (h % 6) + 66]
                nc.vector.reciprocal(rh[:32], src[:, 64:65])
                nc.vector.tensor_scalar_mul(
                    o_nt[:32, HD * h:HD * (h + 1)], src[:, :HD], rh[:32])

            # ====== tail 32 queries: LN + transpose + output projection =====
            if phases >= 4:
                ln_stats(o_nt[:32], 32, 4)
                nc.scalar.activation(
                    rstd[:, 4:5], mva[:, 4:5, 1], AF.Sqrt, bias=eps_t)
                nc.vector.reciprocal(rstd[:, 4:5], rstd[:, 4:5])
                ln_apply(o_nt[:32], 32, 4)
            for fc in range(DC if phases >= 5 else 0):
                tp = projp.tile([128, 32], BF16, tag="proj", name=f"tpt{fc}")
                nc.tensor.transpose(
                    tp[:, :32], o_nt[:32, 128 * fc:128 * (fc + 1)],
                    ident[:32, :32])
                nc.vector.tensor_copy(o_lnT[:, fc, MQ:TQ], tp[:, :32])
            if phases >= 5:
                ou_t = sml.tile([128, DC, 32], FP32, tag="out", name="out_t")
            for fc in range(DC if phases >= 5 else 0):
                for si in (1, 2):
                    c0, n, wo, bo = o_segs[si]
                    pu = prj()
                    for dc in range(DC):
                        nc.tensor.matmul(
                            pu[:, :n],
                            wo[:, dc, 128 * fc:128 * (fc + 1)],
                            o_lnT[:, dc, c0:c0 + n],
                            start=(dc == 0), stop=(dc == DC - 1))
                    nc.vector.tensor_scalar_add(
                        ou_t[:, fc, c0 - MQ:c0 - MQ + n], pu[:, :n],
                        bo[:, fc:fc + 1])
            if phases >= 5:
                nc.sync.dma_start(outT_v[:, :, MQ:TQ], ou_t)

    nc.compile()
    return nc


def _run_spmd_dedup(nc, shared, percore):
    """Dispatch the prebuilt Bass module on 8 cores via PJRT.

    Shared inputs are uploaded sharded (1x wire traffic) and replicated
    on-device; donated output buffers are created on-device. Device-resident
    replicas are cached by content hash across calls."""
    import zlib
    import jax
    import jax.numpy as jnp
    from jax.experimental.shard_map import shard_map
    from jax.sharding import Mesh, PartitionSpec as P, NamedSharding
    from concourse import bass2jax, mybir

    bass2jax.install_neuronx_cc_hook()
    partition_name = (nc.partition_id_tensor.name
                      if nc.partition_id_tensor else None)
    in_names, out_names, out_avals = [], [], []
    for alloc in nc.m.functions[0].allocations:
        if not isinstance(alloc, mybir.MemoryLocationSet):
            continue
        name = alloc.memorylocations[0].name
        if alloc.kind == "ExternalInput":
            if name != partition_name:
                in_names.append(name)
        elif alloc.kind == "ExternalOutput":
            out_names.append(name)
            shape = tuple(alloc.tensor_shape)
            out_avals.append(jax.core.ShapedArray(shape, mybir.dt.np(alloc.dtype)))
    n_params = len(in_names)
    all_names = in_names + out_names
    if partition_name is not None:
        all_names = all_names + [partition_name]

    def _body(*args):
        ops = list(args)
        if partition_name is not None:
            ops.append(bass2jax.partition_id_tensor())
        outs = bass2jax._bass_exec_p.bind(
            *ops, out_avals=tuple(out_avals), in_names=tuple(all_names),
            out_names=tuple(out_names), lowering_input_output_aliases=(),
            sim_require_finite=True, sim_require_nnan=True, nc=nc)
        return tuple(outs)

    devices = jax.devices()[:NCORES]
    mesh = Mesh(np.asarray(devices), ("core",))
    rep = NamedSharding(mesh, P(None))
    shd = NamedSharding(mesh, P("core"))
    in_specs = tuple(P(None) if n in shared else P("core") for n in in_names) \
        + (P("core"),) * len(out_names)
    out_specs = (P("core"),) * len(out_names)
    donate = tuple(range(n_params, n_params + len(out_names)))
    if "jit_fn" not in _CACHE:
        _CACHE["jit_fn"] = jax.jit(
            shard_map(_body, mesh=mesh, in_specs=in_specs,
                      out_specs=out_specs, check_rep=False),
            donate_argnums=donate, keep_unused=True)
        _CACHE["replicate"] = jax.jit(lambda a: a, out_shardings=rep)
        _CACHE["dev_cache"] = {}

    def dev_shared(name, arr):
        key = (name, arr.shape, zlib.adler32(arr.tobytes()))
        c = _CACHE["dev_cache"]
        if c.get(name, (None, None))[0] == key:
            return c[name][1]
        a_sh = jax.device_put(arr, shd)        # 1x wire traffic
        a_rep = _CACHE["replicate"](a_sh)      # on-device all-gather
        c[name] = (key, a_rep)
        return a_rep

    zeros_fn = _CACHE.setdefault("zeros_fn", jax.jit(
        lambda: tuple(jnp.zeros((NCORES * a.shape[0], *a.shape[1:]), a.dtype)
                      for a in out_avals),
        out_shardings=tuple(shd for _ in out_avals)))

    ins = [dev_shared(n, shared[n]) if n in shared else
           jax.device_put(np.concatenate(percore[n], axis=0), shd)
           for n in in_names]
    zouts = zeros_fn()
    out_arrs = _CACHE["jit_fn"](*ins, *zouts)
    return [
        {name: np.asarray(out_arrs[i]).reshape(NCORES, *out_avals[i].shape)[c]
         for i, name in enumerate(out_names)}
        for c in range(NCORES)
    ]


def kernel(**inputs):
    import ml_dtypes
    from concourse import bass_utils

    BF = ml_dtypes.bfloat16

    if "nc" not in _CACHE:
        _CACHE["nc"] = _build()
    nc = _CACHE["nc"]

    f = {k: np.ascontiguousarray(np.asarray(v, dtype=np.float32))
         for k, v in inputs.items()}
    x = f["x"][0]                                   # [4301, 768]
    xT = np.ascontiguousarray(x.T)                  # [768, 4301]
    xTp = np.zeros((D, NPAD), BF)
    xTp[:, :N_TOK] = xT.astype(BF)

    base = {
        "xT": xTp,
        "wqT_p": np.ascontiguousarray(f["wq_p"].T.astype(BF)),
        "wqT_d": np.ascontiguousarray(f["wq_d"].T.astype(BF)),
        "wkT_p": np.ascontiguousarray(f["wk_p"].T.astype(BF)),
        "wkT_d": np.ascontiguousarray(f["wk_d"].T.astype(BF)),
        "wvT_p": np.ascontiguousarray(f["wv_p"].T.astype(BF)),
        "wvT_d": np.ascontiguousarray(f["wv_d"].T.astype(BF)),
        "woT_p": np.ascontiguousarray(f["wo_p"].T.astype(BF)),
        "woT_d": np.ascontiguousarray(f["wo_d"].T.astype(BF)),
        "bq_p": f["bq_p"], "bq_d": f["bq_d"],
        "bv_p": f["bv_p"], "bv_d": f["bv_d"],
        "bo_p": f["bo_p"], "bo_d": f["bo_d"],
        "lng": f["ln_g"].astype(BF), "lnb": f["ln_b"].astype(BF),
    }
    in_maps = []
    for c in range(NCORES):
        xqT = np.zeros((D, TQ), BF)
        p0, p1 = PQ * c, min(PQ * (c + 1), NPATCH)
        if p1 > p0:
            xqT[:, :p1 - p0] = xT[:, p0:p1].astype(BF)
        d0, d1 = DQ * c, min(DQ * (c + 1), NDET)
        if d1 > d0:
            xqT[:, PQ:PQ + d1 - d0] = xT[:, NPATCH + d0:NPATCH + d1].astype(BF)
        in_maps.append({**base, "xqT": np.ascontiguousarray(xqT)})

    try:
        results = _run_spmd_dedup(
            nc, shared=base,
            percore={"xqT": [m["xqT"] for m in in_maps]})
    except Exception:
        _CACHE.pop("jit_fn", None)
        results = bass_utils.run_bass_kernel_spmd(
            nc, in_maps, core_ids=list(range(NCORES))).results

    out = np.empty((N_TOK, D), np.float32)
    for c in range(NCORES):
        oc = results[c]["outT"].T                   # [544, 768]
        p0, p1 = PQ * c, min(PQ * (c + 1), NPATCH)
        if p1 > p0:
            out[p0:p1] = oc[:p1 - p0]
        d0, d1 = DQ * c, min(DQ * (c + 1), NDET)
        if d1 > d0:
            out[NPATCH + d0:NPATCH + d1] = oc[PQ:PQ + d1 - d0]
    return out[None]

